# revision 1
# baseline (speedup 1.0000x reference)
"""Multi-head self-attention (CogView PB-relax variant) on 8 TRN2 NeuronCores.

Problem: B=2, S=2048, D=1024, H=16 heads, Dh=64.
  q/k/v = hidden @ W{q,k,v}.T + b          (per-head slices)
  scores = (q k^T + attn_bias) / 8 + (1-mask)*(-BIG)
  out    = softmax(scores) @ v             (PB-relax softmax == plain softmax)

Sharding: tensor-parallel over heads. Core c owns heads (2c, 2c+1) for both
batch rows: it reads full hidden, W-row slices [128c:128c+128], bias slice
[h=2c:2c+2], and writes output channels [128c:128(c+1)].

Device-side design (v7):
  - the host pre-transposes / pre-casts the raw inputs when building the
    per-core in_maps (pure layout work): hidden^T, W^T and bias^T arrive as
    bf16 DRAM tensors in exactly the layouts the matmuls want. No on-device
    transposes or casts remain except the tiny V^T->V xbar.
  - phase 1: q^T/k^T/v^T projections (bf16 matmuls, contraction=D tiled by
    128); q^T/k^T kept [head-dim, token] resident in SBUF.
  - phase 2, per (q-block, batch, k-chunk): scores computed TRANSPOSED
    [k=128, q=512] in PSUM (contraction 64, both heads packed in the PE
    array via tile_position row groups); the vector engine adds bias^T and
    drains PSUM to SBUF; ACT computes exp(x*0.125 + maskbias[k]) for both
    heads in one call (the per-partition maskbias column applies the
    attention mask for free); AV accumulates ctx^T with lhsT = [v | 1]
    (65 cols) so row 64 is the masked softmax denominator.
  - epilogue: PE-transpose back to [q, d] (f32 exact), per-partition
    reciprocal, scale, store.
"""

import numpy as np
import ml_dtypes

import concourse.bass as bass
import concourse.mybir as mybir
import concourse.tile as tile
from concourse import bacc, bass_utils
from concourse.masks import make_identity

F32 = mybir.dt.float32
BF16 = mybir.dt.bfloat16
I32 = mybir.dt.int32
Exp = mybir.ActivationFunctionType.Exp

B, S, D = 2, 2048, 1024
NCORES = 8
HPC = 2            # heads per core
OC = HPC * 64      # 128 output channels per core
QB = 512           # q block (free dim of score tiles)
NQB = S // QB      # 4
NKC = S // 128     # 16 k-chunks per batch row
NSB = (B * S) // 512   # 8 token blocks for projections
NDC = D // 128     # 8 contraction chunks

MASK_NEG = -30000.0
SCALE = 0.125


def _build_program():
    nc = bacc.Bacc(
        "TRN2", target_bir_lowering=False, debug=False, num_devices=NCORES
    )
    hidT = nc.dram_tensor("hid_t", [D, B * S], BF16, kind="ExternalInput").ap()
    amask = nc.dram_tensor("attention_mask", [B, S], I32, kind="ExternalInput").ap()
    biasT = nc.dram_tensor("bias_t", [HPC, S, S], BF16, kind="ExternalInput").ap()
    wqt = nc.dram_tensor("wq_t", [D, OC], BF16, kind="ExternalInput").ap()
    wkt = nc.dram_tensor("wk_t", [D, OC], BF16, kind="ExternalInput").ap()
    wvt = nc.dram_tensor("wv_t", [D, OC], BF16, kind="ExternalInput").ap()
    bq = nc.dram_tensor("bq", [OC], F32, kind="ExternalInput").ap()
    bk = nc.dram_tensor("bk", [OC], F32, kind="ExternalInput").ap()
    bv = nc.dram_tensor("bv", [OC], F32, kind="ExternalInput").ap()
    out = nc.dram_tensor("out", [B, S, OC], F32, kind="ExternalOutput").ap()

    with tile.TileContext(nc) as tc:
        _attention(tc, out, hidT, amask, biasT,
                   [wqt, wkt, wvt], [bq, bk, bv])

    nc.compile()
    return nc


def _attention(tc, out, hidT, amask, biasT, ws, bs):
    nc = tc.nc

    with tc.tile_pool(name="singles", bufs=1) as singles:
        ident = singles.tile([128, 128], F32)    # for epilogue PE transposes
        make_identity(nc, ident)
        identb = singles.tile([128, 128], BF16)  # for PE bias-inject matmuls
        make_identity(nc, identb)

        # --- mask -> additive bias column layout [128, B, NKC] ------------
        mi = singles.tile([128, B, NKC], I32)
        nc.gpsimd.dma_start(out=mi, in_=amask.rearrange("b (c p) -> p b c", p=128))
        mf = singles.tile([128, B, NKC], F32)
        nc.vector.tensor_copy(out=mf, in_=mi)
        mb = singles.tile([128, B, NKC], F32)
        nc.vector.tensor_scalar(
            out=mb, in0=mf, scalar1=-MASK_NEG, scalar2=MASK_NEG,
            op0=mybir.AluOpType.mult, op1=mybir.AluOpType.add,
        )

        # --- projection bias vectors [128, 1] -----------------------------
        bvec = []
        for i, b_ap in enumerate(bs):
            t = singles.tile([128, 1], F32, tag=f"bvec{i}")
            nc.gpsimd.dma_start(out=t, in_=b_ap.rearrange("(p o) -> p o", o=1))
            bvec.append(t)

        ones_col = singles.tile([128, 1], BF16)
        nc.vector.memset(ones_col, 1.0)

        # --- W^T tiles [d-local, dc, o] straight from DRAM ----------------
        wt3 = []
        for i, w_ap in enumerate(ws):
            t = singles.tile([128, NDC, 128], BF16, tag=f"wt{i}")
            nc.sync.dma_start(
                out=t, in_=w_ap.rearrange("(c p) o -> p c o", p=128))
            wt3.append(t)

        # --- persistent activations (bf16) --------------------------------
        qt2 = singles.tile([128, B * S], BF16, tag="qt2")
        kt2 = singles.tile([128, B * S], BF16, tag="kt2")
        va = singles.tile([128, 2 * NKC, 2 * 66], BF16, tag="va")

        # ============ phase 1: projections ================================
        with tc.tile_pool(name="h_t", bufs=3) as htp, \
             tc.tile_pool(name="v_t", bufs=3) as vtp, \
             tc.tile_pool(name="p_ps", bufs=4, space="PSUM") as pps:
            pend_vt2 = []
            for sb in range(NSB):
                hts = htp.tile([128, NDC, 512], BF16, name="hts")
                nc.sync.dma_start(
                    out=hts,
                    in_=hidT[:, sb * 512:(sb + 1) * 512]
                    .rearrange("(c p) s -> p c s", p=128))
                for w in range(3):
                    pp = pps.tile([128, 512], F32)
                    for dc in range(NDC):
                        nc.tensor.matmul(
                            out=pp,
                            lhsT=wt3[w][:, dc, :],
                            rhs=hts[:, dc, :],
                            start=(dc == 0), stop=(dc == NDC - 1))
                    if w < 2:
                        dst = (qt2 if w == 0 else kt2)[:, sb * 512:(sb + 1) * 512]
                        nc.scalar.activation(
                            out=dst, in_=pp,
                            func=mybir.ActivationFunctionType.Identity,
                            bias=bvec[w])
                    else:
                        if sb % 2 == 0:
                            vt2 = vtp.tile([128, 2, 512], BF16, name="vt2")
                            pend_vt2.append(vt2)
                        else:
                            vt2 = pend_vt2[-1]
                        nc.vector.tensor_scalar_add(
                            out=vt2[:, sb % 2, :], in0=pp, scalar1=bvec[2])
                        if sb % 2 == 1:
                            vts = vtp.tile([128, 8, 128], BF16, name="vts")
                            nc.sync.dma_start(
                                out=vts, in_=vt2.rearrange("p j q -> p (j q)"),
                                transpose=True)
                            for j in range(8):
                                kb = (sb - 1) * 4 + j
                                for h in range(HPC):
                                    nc.gpsimd.tensor_copy(
                                        out=va[:, kb, h * 66:h * 66 + 64],
                                        in_=vts[:, j, h * 64:(h + 1) * 64])
                                    nc.gpsimd.tensor_copy(
                                        out=va[:, kb, h * 66 + 64:h * 66 + 65],
                                        in_=ones_col)

        # ============ phase 2: attention ==================================
        with tc.tile_pool(name="b_t", bufs=4) as btp, \
             tc.tile_pool(name="pt", bufs=12) as ptp, \
             tc.tile_pool(name="se", bufs=10) as sep, \
             tc.tile_pool(name="stage", bufs=3) as stp, \
             tc.tile_pool(name="osb", bufs=3) as osp, \
             tc.tile_pool(name="sc_ps", bufs=4, space="PSUM") as scp, \
             tc.tile_pool(name="ctx_ps", bufs=4, space="PSUM") as cxp:
            for qb in range(NQB):
                ctx = [[cxp.tile([65, QB], F32, tag="ctx", name=f"ctx{b}{h}")
                        for h in range(HPC)] for b in range(B)]
                # bias^T [k, q-block] straight from DRAM, per head
                bt = []
                for h in range(HPC):
                    t = btp.tile([128, NKC, QB], BF16, tag="bT", name=f"bt{h}")
                    nc.sync.dma_start(
                        out=t,
                        in_=biasT[h, :, qb * QB:(qb + 1) * QB]
                        .rearrange("(c p) q -> p c q", p=128))
                    bt.append(t)
                for b in range(B):
                    for kc in range(NKC):
                        pe_inject = False
                        scs = []
                        for h in range(HPC):
                            sc = scp.tile([128, QB], F32, tag="sc", name="sc")
                            if pe_inject:
                                nc.tensor.matmul(
                                    out=sc, lhsT=identb,
                                    rhs=bt[h][:, kc, :],
                                    start=True, stop=False,
                                    skip_group_check=True)
                            nc.tensor.matmul(
                                out=sc,
                                lhsT=kt2[h * 64:(h + 1) * 64,
                                         b * S + kc * 128:
                                         b * S + (kc + 1) * 128],
                                rhs=qt2[h * 64:(h + 1) * 64,
                                        b * S + qb * QB:
                                        b * S + (qb + 1) * QB],
                                start=not pe_inject, stop=True,
                                tile_position=(h * 64, 0),
                                skip_group_check=True)
                            scs.append(sc)
                        pt = ptp.tile([128, HPC, QB], BF16, tag="pt", name="pt")
                        if pe_inject:
                            # exp reads PSUM directly, one call per head
                            for h in range(HPC):
                                nc.scalar.activation(
                                    out=pt[:, h, :], in_=scs[h], func=Exp,
                                    bias=mb[:, b, kc:kc + 1], scale=SCALE)
                        else:
                            # bias add on DVE drains PSUM into SBUF
                            se = sep.tile([128, HPC, QB], F32, tag="se", name="se")
                            for h in range(HPC):
                                nc.vector.tensor_tensor(
                                    out=se[:, h, :], in0=scs[h],
                                    in1=bt[h][:, kc, :],
                                    op=mybir.AluOpType.add)
                            nc.scalar.activation(
                                out=pt.rearrange("p h q -> p (h q)"),
                                in_=se.rearrange("p h q -> p (h q)"), func=Exp,
                                bias=mb[:, b, kc:kc + 1], scale=SCALE)
                        for h in range(HPC):
                            nc.tensor.matmul(
                                out=ctx[b][h],
                                lhsT=va[:, b * NKC + kc,
                                        h * 66:h * 66 + 65],
                                rhs=pt[:, h, :],
                                start=(kc == 0), stop=(kc == NKC - 1))
                    # ---- epilogue: normalize, transpose, store -----------
                    stage = stp.tile([128, QB], F32, tag="stage", name="stage")
                    rst = stp.tile([128, QB], F32, tag="rst", name="rst")
                    for h in range(HPC):
                        # ctx drain on ACT (idle) instead of the saturated DVE
                        nc.scalar.activation(
                            out=stage[h * 64:(h + 1) * 64, :],
                            in_=ctx[b][h][0:64, :],
                            func=mybir.ActivationFunctionType.Copy)
                        nc.vector.tensor_copy(
                            out=rst[32 * h:32 * h + 1, :],
                            in_=ctx[b][h][64:65, :])
                    osb = osp.tile([128, 4, 128], F32, tag="osb", name="osb")
                    for i in range(4):
                        tp = scp.tile([128, 128], F32, tag="sc", name="ep_t")
                        rp = scp.tile([128, 128], F32, tag="sc", name="ep_r")
                        nc.tensor.transpose(
                            out=tp, in_=stage[:, i * 128:(i + 1) * 128],
                            identity=ident)
                        nc.tensor.transpose(
                            out=rp, in_=rst[:, i * 128:(i + 1) * 128],
                            identity=ident)
                        rcp = stp.tile([128, 2], F32, tag="rcp", name="rcp")
                        for h in range(HPC):
                            nc.vector.reciprocal(
                                out=rcp[:, h:h + 1],
                                in_=rp[:, 32 * h:32 * h + 1])
                            nc.vector.tensor_scalar_mul(
                                out=osb[:, i, h * 64:(h + 1) * 64],
                                in0=tp[:, h * 64:(h + 1) * 64],
                                scalar1=rcp[:, h:h + 1])
                    nc.gpsimd.dma_start(
                        out=out[b, qb * QB:(qb + 1) * QB, :]
                        .rearrange("(i p) k -> p i k", p=128),
                        in_=osb)


_CACHE = {}


def _get_program():
    if "nc" not in _CACHE:
        _CACHE["nc"] = _build_program()
    return _CACHE["nc"]


def _shard_inputs(inputs):
    """Host-side layout prep: transposes and bf16 casts only (no compute)."""
    bf = ml_dtypes.bfloat16
    hs = np.asarray(inputs["hidden_state"], dtype=np.float32)
    hid_t = np.ascontiguousarray(hs.reshape(B * S, D).T).astype(bf)   # [D, B*S]
    am = np.ascontiguousarray(np.asarray(inputs["attention_mask"], dtype=np.int32))
    ab = np.asarray(inputs["attention_bias"], dtype=np.float32)
    wts = {k: np.asarray(inputs[k], dtype=np.float32) for k in ("Wq", "Wk", "Wv")}
    vb = {k: np.ascontiguousarray(np.asarray(inputs[k], dtype=np.float32))
          for k in ("bq", "bk", "bv")}
    in_maps = []
    for c in range(NCORES):
        r0, r1 = c * OC, (c + 1) * OC
        bias_t = np.ascontiguousarray(
            ab[0, HPC * c:HPC * (c + 1)].transpose(0, 2, 1)).astype(bf)
        in_maps.append({
            "hid_t": hid_t,
            "attention_mask": am,
            "bias_t": bias_t,                                   # [h, k, q]
            "wq_t": np.ascontiguousarray(wts["Wq"][r0:r1].T).astype(bf),
            "wk_t": np.ascontiguousarray(wts["Wk"][r0:r1].T).astype(bf),
            "wv_t": np.ascontiguousarray(wts["Wv"][r0:r1].T).astype(bf),
            "bq": vb["bq"][r0:r1],
            "bk": vb["bk"][r0:r1],
            "bv": vb["bv"][r0:r1],
        })
    return in_maps


def kernel(**inputs):
    nc = _get_program()
    in_maps = _shard_inputs(inputs)
    res = bass_utils.run_bass_kernel_spmd(
        nc, in_maps, core_ids=list(range(NCORES)))
    parts = [np.asarray(res.results[c]["out"]) for c in range(NCORES)]
    return np.concatenate(parts, axis=-1)


def run_profiled(inputs, trace=True):
    """test.py helper: returns (output, BassKernelResults)."""
    nc = _get_program()
    in_maps = _shard_inputs(inputs)
    res = bass_utils.run_bass_kernel_spmd(
        nc, in_maps, core_ids=list(range(NCORES)), trace=trace)
    parts = [np.asarray(res.results[c]["out"]) for c in range(NCORES)]
    return np.concatenate(parts, axis=-1), res



# revision 15
# speedup vs baseline: 1.1375x; 1.1375x over previous
"""Multi-head self-attention (CogView PB-relax variant) on 8 TRN2 NeuronCores.

Problem: B=2, S=2048, D=1024, H=16 heads, Dh=64.
  q/k/v = hidden @ W{q,k,v}.T + b          (per-head slices)
  scores = (q k^T + attn_bias) / 8 + (1-mask)*(-BIG)
  out    = softmax(scores) @ v             (PB-relax softmax == plain softmax)

Sharding: tensor-parallel over heads. Core c owns heads (2c, 2c+1) for both
batch rows: it reads full hidden, W-row slices [128c:128c+128], bias slice
[h=2c:2c+2], and writes output channels [128c:128(c+1)].

Device-side design (v8c):
  The ACT (scalar) engine is the hard floor: it must exp() every score
  element (16.8M per core, ~1 col/cycle). Everything else is arranged so
  ACT does ONLY exp and never starves:
  - the bias add is split between PE and DVE to balance both near ACT's
    floor: for PE units the bias is "injected" into PSUM by an fp8
    identity matmul (start=True) and the bf16 score matmul accumulates
    onto it (start=False) — HW-probed legal (fp8 DoubleRow inject in the
    same group hangs the PE; plain fp8 works). For DVE units the score
    matmul runs alone and DVE adds the fp8 bias while draining to SBUF.
  - scores stay bf16 (fp8 q/k was measured at rel 2.6e-2 — over the gate),
    both heads packed in the PE array via tile_position row groups.
  - exp handles [128, 2 heads, 512 q] in one ACT call (PSUM source for PE
    units, SBUF for DVE units) with per-partition mask bias + 1/8 scale.
  - AV stays bf16 with the [v | 1] augmented lhsT (row 64 = denominator).
  - emission is software-pipelined: unit kc emits inject/score(kc),
    bias-add(kc), exp(kc), then AV(kc-1); each block's epilogue is
    emitted spread across the next block's units so the DVE burst never
    starves ACT.
  - bias travels fp8e4 end-to-end (half DMA), quantization harmless /8.
"""

import numpy as np
import ml_dtypes

import concourse.bass as bass
import concourse.mybir as mybir
import concourse.tile as tile
from concourse import bacc, bass_utils
from concourse.masks import make_identity

F32 = mybir.dt.float32
BF16 = mybir.dt.bfloat16
FP8 = mybir.dt.float8e4
I32 = mybir.dt.int32
Exp = mybir.ActivationFunctionType.Exp
Ident = mybir.ActivationFunctionType.Identity
DR = mybir.MatmulPerfMode.DoubleRow

B, S, D = 2, 2048, 1024
NCORES = 8
HPC = 2            # heads per core
OC = HPC * 64      # 128 output channels per core
QB = 512           # q block (free dim of score tiles)
NQB = S // QB      # 4
NKC = S // 128     # 16 k-chunks per batch row
NSB = (B * S) // 512   # 8 token blocks for projections
NDC = D // 128     # 8 contraction chunks

MASK_NEG = -30000.0
SCALE = 0.125
PE_KCS = frozenset((0, 2, 4, 6, 8, 10, 12))   # units bias-injected on PE


def _build_program():
    nc = bacc.Bacc(
        "TRN2", target_bir_lowering=False, debug=False, num_devices=NCORES
    )
    hidT = nc.dram_tensor("hid_t", [D, B * S], BF16, kind="ExternalInput").ap()
    amask = nc.dram_tensor("attention_mask", [B, S], I32, kind="ExternalInput").ap()
    bias8 = nc.dram_tensor(
        "bias8", [NQB, 128, HPC, NKC, QB], FP8, kind="ExternalInput").ap()
    wqt = nc.dram_tensor("wq_t", [D, OC], BF16, kind="ExternalInput").ap()
    wkt = nc.dram_tensor("wk_t", [D, OC], BF16, kind="ExternalInput").ap()
    wvt = nc.dram_tensor("wv_t", [D, OC], BF16, kind="ExternalInput").ap()
    bq = nc.dram_tensor("bq", [OC], F32, kind="ExternalInput").ap()
    bk = nc.dram_tensor("bk", [OC], F32, kind="ExternalInput").ap()
    bv = nc.dram_tensor("bv", [OC], F32, kind="ExternalInput").ap()
    out = nc.dram_tensor("out", [B, S, OC], F32, kind="ExternalOutput").ap()

    with tile.TileContext(nc) as tc:
        _attention(tc, out, hidT, amask, bias8,
                   [wqt, wkt, wvt], [bq, bk, bv])

    nc.compile()
    return nc


def _attention(tc, out, hidT, amask, bias8, ws, bs):
    nc = tc.nc

    with tc.tile_pool(name="singles", bufs=1) as singles:
        ident = singles.tile([128, 128], F32)    # for epilogue PE transposes
        make_identity(nc, ident)
        id8 = singles.tile([128, 128], FP8)      # fp8 identity for bias inject
        nc.vector.tensor_copy(out=id8, in_=ident)

        # --- mask -> additive bias column layout [128, B, NKC] ------------
        mi = singles.tile([128, B, NKC], I32)
        nc.gpsimd.dma_start(out=mi, in_=amask.rearrange("b (c p) -> p b c", p=128))
        mf = singles.tile([128, B, NKC], F32)
        nc.vector.tensor_copy(out=mf, in_=mi)
        mb = singles.tile([128, B, NKC], F32)
        nc.vector.tensor_scalar(
            out=mb, in0=mf, scalar1=-MASK_NEG, scalar2=MASK_NEG,
            op0=mybir.AluOpType.mult, op1=mybir.AluOpType.add,
        )

        # --- projection bias vectors [128, 1] -----------------------------
        bvec = []
        for i, b_ap in enumerate(bs):
            t = singles.tile([128, 1], F32, tag=f"bvec{i}")
            nc.gpsimd.dma_start(out=t, in_=b_ap.rearrange("(p o) -> p o", o=1))
            bvec.append(t)

        ones_col = singles.tile([128, 1], BF16)
        nc.vector.memset(ones_col, 1.0)

        # --- W^T tiles [d-local, dc, o] straight from DRAM ----------------
        wt3 = []
        for i, w_ap in enumerate(ws):
            t = singles.tile([128, NDC, 128], BF16, tag=f"wt{i}")
            nc.sync.dma_start(
                out=t, in_=w_ap.rearrange("(c p) o -> p c o", p=128))
            wt3.append(t)

        # --- persistent activations (bf16) --------------------------------
        qt2 = singles.tile([128, B * S], BF16, tag="qt2")
        kt2 = singles.tile([128, B * S], BF16, tag="kt2")
        va = singles.tile([128, 2 * NKC, 2 * 66], BF16, tag="va")

        # ============ phase 1: projections ================================
        with tc.tile_pool(name="h_t", bufs=3) as htp, \
             tc.tile_pool(name="v_t", bufs=3) as vtp, \
             tc.tile_pool(name="p_ps", bufs=4, space="PSUM") as pps:
            pend_vt2 = []
            for sb in range(NSB):
                hts = htp.tile([128, NDC, 512], BF16, name="hts")
                nc.sync.dma_start(
                    out=hts,
                    in_=hidT[:, sb * 512:(sb + 1) * 512]
                    .rearrange("(c p) s -> p c s", p=128))
                for w in range(3):
                    pp = pps.tile([128, 512], F32)
                    for dc in range(NDC):
                        nc.tensor.matmul(
                            out=pp,
                            lhsT=wt3[w][:, dc, :],
                            rhs=hts[:, dc, :],
                            start=(dc == 0), stop=(dc == NDC - 1))
                    if w < 2:
                        dst = (qt2 if w == 0 else kt2)[:, sb * 512:(sb + 1) * 512]
                        nc.scalar.activation(
                            out=dst, in_=pp, func=Ident, bias=bvec[w])
                    else:
                        if sb % 2 == 0:
                            vt2 = vtp.tile([128, 2, 512], BF16, name="vt2")
                            pend_vt2.append(vt2)
                        else:
                            vt2 = pend_vt2[-1]
                        nc.vector.tensor_scalar_add(
                            out=vt2[:, sb % 2, :], in0=pp, scalar1=bvec[2])
                        if sb % 2 == 1:
                            vts = vtp.tile([128, 8, 128], BF16, name="vts")
                            nc.sync.dma_start(
                                out=vts, in_=vt2.rearrange("p j q -> p (j q)"),
                                transpose=True)
                            for j in range(8):
                                kb = (sb - 1) * 4 + j
                                for h in range(HPC):
                                    nc.gpsimd.tensor_copy(
                                        out=va[:, kb, h * 66:h * 66 + 64],
                                        in_=vts[:, j, h * 64:(h + 1) * 64])
                                    nc.gpsimd.tensor_copy(
                                        out=va[:, kb, h * 66 + 64:h * 66 + 65],
                                        in_=ones_col)

        # ============ phase 2: attention ==================================
        with tc.tile_pool(name="b_t", bufs=2) as btp, \
             tc.tile_pool(name="pt", bufs=3) as ptp, \
             tc.tile_pool(name="se", bufs=3) as sep, \
             tc.tile_pool(name="stage", bufs=3) as stp, \
             tc.tile_pool(name="osb", bufs=3) as osp, \
             tc.tile_pool(name="sc_ps", bufs=2, space="PSUM") as scp, \
             tc.tile_pool(name="ep_ps", bufs=2, space="PSUM") as epp, \
             tc.tile_pool(name="ctx_ps", bufs=2, space="PSUM") as cxp:
            bt8s = {}
            bt8s[0] = btp.tile([128, HPC, NKC, QB], FP8, tag="bt8",
                               name="bt8_0")
            nc.sync.dma_start(out=bt8s[0], in_=bias8[0])
            pending = []   # deferred epilogue emitters, drained 1/unit

            def emit_epilogue(ctx, b, qb):
                stage = stp.tile([128, QB], F32, tag="stage", name="stage")
                rst = stp.tile([128, QB], F32, tag="rst", name="rst")
                osb = osp.tile([128, 4, 128], F32, tag="osb", name="osb")

                def head_drain():
                    for h in range(HPC):
                        nc.vector.tensor_copy(
                            out=stage[h * 64:(h + 1) * 64, :],
                            in_=ctx[h][0:64, :])
                        nc.vector.tensor_copy(
                            out=rst[32 * h:32 * h + 1, :],
                            in_=ctx[h][64:65, :])
                pending.append(head_drain)

                def quarter(i):
                    def emit():
                        tp = epp.tile([128, 128], F32, tag="ep", name="ep_t")
                        rp = epp.tile([128, 128], F32, tag="ep", name="ep_r")
                        nc.tensor.transpose(
                            out=tp, in_=stage[:, i * 128:(i + 1) * 128],
                            identity=ident)
                        nc.tensor.transpose(
                            out=rp, in_=rst[:, i * 128:(i + 1) * 128],
                            identity=ident)
                        rcp = stp.tile([128, 2], F32, tag="rcp", name="rcp")
                        for h in range(HPC):
                            nc.vector.reciprocal(
                                out=rcp[:, h:h + 1],
                                in_=rp[:, 32 * h:32 * h + 1])
                            nc.vector.tensor_scalar_mul(
                                out=osb[:, i, h * 64:(h + 1) * 64],
                                in0=tp[:, h * 64:(h + 1) * 64],
                                scalar1=rcp[:, h:h + 1])
                    return emit
                for i in range(4):
                    pending.append(quarter(i))

                def store():
                    nc.gpsimd.dma_start(
                        out=out[b, qb * QB:(qb + 1) * QB, :]
                        .rearrange("(i p) k -> p i k", p=128),
                        in_=osb)
                pending.append(store)

            for qb in range(NQB):
                bt8 = bt8s.pop(qb)
                for b in range(B):
                    if b == 1 and qb + 1 < NQB:
                        # prefetch next q-block's bias during this block
                        nxt = btp.tile([128, HPC, NKC, QB], FP8, tag="bt8",
                                       name=f"bt8_{qb+1}")
                        nc.sync.dma_start(out=nxt, in_=bias8[qb + 1])
                        bt8s[qb + 1] = nxt
                    ctx = [cxp.tile([65, QB], F32, tag="ctx", name=f"ctx{b}{h}")
                           for h in range(HPC)]
                    prev_pt = None

                    def emit_av(pt, kc, ctx=ctx, b=b):
                        for h in range(HPC):
                            nc.tensor.matmul(
                                out=ctx[h],
                                lhsT=va[:, b * NKC + kc, h * 66:h * 66 + 65],
                                rhs=pt[:, h, :],
                                start=(kc == 0), stop=(kc == NKC - 1))

                    for kc in range(NKC):
                        pe_unit = kc in PE_KCS
                        sc2 = scp.tile([128, HPC, QB], F32, tag="sc", name="sc2")
                        for h in range(HPC):
                            if pe_unit:
                                nc.tensor.matmul(
                                    out=sc2[:, h, :], lhsT=id8,
                                    rhs=bt8[:, h, kc, :],
                                    start=True, stop=False,
                                    skip_group_check=True)
                            nc.tensor.matmul(
                                out=sc2[:, h, :],
                                lhsT=kt2[h * 64:(h + 1) * 64,
                                         b * S + kc * 128:
                                         b * S + (kc + 1) * 128],
                                rhs=qt2[h * 64:(h + 1) * 64,
                                        b * S + qb * QB:
                                        b * S + (qb + 1) * QB],
                                start=not pe_unit, stop=True,
                                tile_position=(h * 64, 0),
                                skip_group_check=True)
                        if pe_unit:
                            esrc = sc2
                        else:
                            esrc = sep.tile([128, HPC, QB], F32,
                                            tag="se", name="se")
                            for h in range(HPC):
                                nc.vector.tensor_tensor(
                                    out=esrc[:, h, :], in0=sc2[:, h, :],
                                    in1=bt8[:, h, kc, :],
                                    op=mybir.AluOpType.add)
                        pt = ptp.tile([128, HPC, QB], BF16, tag="pt", name="pt")
                        nc.scalar.activation(
                            out=pt.rearrange("p h q -> p (h q)"),
                            in_=esrc.rearrange("p h q -> p (h q)"), func=Exp,
                            bias=mb[:, b, kc:kc + 1], scale=SCALE)
                        if prev_pt is not None:
                            emit_av(*prev_pt)
                        if pending:
                            pending.pop(0)()
                        prev_pt = (pt, kc)
                    emit_av(*prev_pt)
                    emit_epilogue(ctx, b, qb)
            while pending:
                pending.pop(0)()


_CACHE = {}


def _get_program():
    if "nc" not in _CACHE:
        _CACHE["nc"] = _build_program()
    return _CACHE["nc"]


def _shard_inputs(inputs):
    """Host-side layout prep: transposes and dtype casts only (no compute)."""
    bf = ml_dtypes.bfloat16
    fp8 = ml_dtypes.float8_e4m3fn
    hs = np.asarray(inputs["hidden_state"], dtype=np.float32)
    hid_t = np.ascontiguousarray(hs.reshape(B * S, D).T).astype(bf)   # [D, B*S]
    am = np.ascontiguousarray(np.asarray(inputs["attention_mask"], dtype=np.int32))
    ab = np.asarray(inputs["attention_bias"], dtype=np.float32)
    wts = {k: np.asarray(inputs[k], dtype=np.float32) for k in ("Wq", "Wk", "Wv")}
    vb = {k: np.ascontiguousarray(np.asarray(inputs[k], dtype=np.float32))
          for k in ("bq", "bk", "bv")}
    in_maps = []
    for c in range(NCORES):
        r0, r1 = c * OC, (c + 1) * OC
        # bias8[qb, p, h, kc, q] = ab[0, 2c+h, qb*QB+q, kc*128 + p]
        x = ab[0, HPC * c:HPC * (c + 1)]           # [h, q(S), k(S)]
        x = x.transpose(2, 0, 1)                   # [k, h, q]
        x = x.reshape(NKC, 128, HPC, NQB, QB)      # [kc, p, h, qb, q]
        x = x.transpose(3, 1, 2, 0, 4)             # [qb, p, h, kc, q]
        bias8 = np.ascontiguousarray(x).astype(fp8)
        in_maps.append({
            "hid_t": hid_t,
            "attention_mask": am,
            "bias8": bias8,
            "wq_t": np.ascontiguousarray(wts["Wq"][r0:r1].T).astype(bf),
            "wk_t": np.ascontiguousarray(wts["Wk"][r0:r1].T).astype(bf),
            "wv_t": np.ascontiguousarray(wts["Wv"][r0:r1].T).astype(bf),
            "bq": vb["bq"][r0:r1],
            "bk": vb["bk"][r0:r1],
            "bv": vb["bv"][r0:r1],
        })
    return in_maps


def kernel(**inputs):
    nc = _get_program()
    in_maps = _shard_inputs(inputs)
    res = bass_utils.run_bass_kernel_spmd(
        nc, in_maps, core_ids=list(range(NCORES)))
    parts = [np.asarray(res.results[c]["out"]) for c in range(NCORES)]
    return np.concatenate(parts, axis=-1)


def run_profiled(inputs, trace=True):
    """test.py helper: returns (output, BassKernelResults)."""
    nc = _get_program()
    in_maps = _shard_inputs(inputs)
    res = bass_utils.run_bass_kernel_spmd(
        nc, in_maps, core_ids=list(range(NCORES)), trace=trace)
    parts = [np.asarray(res.results[c]["out"]) for c in range(NCORES)]
    return np.concatenate(parts, axis=-1), res


# revision 18
# speedup vs baseline: 1.4227x; 1.2508x over previous
"""Multi-head self-attention (CogView PB-relax variant) on 8 TRN2 NeuronCores.

Problem: B=2, S=2048, D=1024, H=16 heads, Dh=64.
  q/k/v = hidden @ W{q,k,v}.T + b          (per-head slices)
  scores = (q k^T + attn_bias) / 8 + (1-mask)*(-BIG)
  out    = softmax(scores) @ v             (PB-relax softmax == plain softmax)

Sharding: tensor-parallel over heads. Core c owns heads (2c, 2c+1) for both
batch rows: it reads full hidden, W-row slices [128c:128c+128], bias slice
[h=2c:2c+2], and writes output channels [128c:128(c+1)].

Device-side design (v9):
  The ACT (scalar) engine is the hard floor: it must exp() every score
  element (16.8M per core at ~1 col/cycle ~= 140 us). Everything else is
  arranged to hide under it:
  - batch-outer loop: only b=0's projections run up front; b=1's
    projections are emitted through a feeder queue into b=0's attention
    blocks (one closure per unit) so they fill PE/ACT/DVE idle slots.
  - bias add split between PE and DVE: kc in PE_KCS gets an fp8 identity
    "inject" matmul (start=True) with the bf16 score matmul accumulating
    on top (start=False); other kc run the score matmul alone and DVE
    adds the fp8 bias while draining PSUM->SBUF. PE units are placed at
    kc 0-5 (+15) so each block's epilogue DVE work (also fed one closure
    per unit) lands where DVE is otherwise idle.
  - exp does [128, 2 heads, 512 q] per ACT call (PSUM source for PE
    units, SBUF for DVE units), per-partition mask bias, 1/8 scale.
  - AV stays bf16 with the [v | 1] augmented lhsT (row 64 = denominator).
  - software-pipelined emission per unit: inject/scores(kc), add(kc),
    exp(kc), AV(kc-1), one feeder pop.
  - host pre-arranges hidden/W/bias so each big DMA moves 8-16KB
    contiguous per partition (descriptor-count-bound otherwise).
  - bias travels fp8e4 end-to-end; quantization harmless pre-softmax /8.
"""

import numpy as np
import ml_dtypes

import concourse.bass as bass
import concourse.mybir as mybir
import concourse.tile as tile
from concourse import bacc, bass_utils
from concourse.masks import make_identity

F32 = mybir.dt.float32
BF16 = mybir.dt.bfloat16
FP8 = mybir.dt.float8e4
I32 = mybir.dt.int32
Exp = mybir.ActivationFunctionType.Exp
Ident = mybir.ActivationFunctionType.Identity

B, S, D = 2, 2048, 1024
NCORES = 8
HPC = 2            # heads per core
OC = HPC * 64      # 128 output channels per core
QB = 512           # q block (free dim of score tiles)
NQB = S // QB      # 4
NKC = S // 128     # 16 k-chunks per batch row
NSB = (B * S) // 512   # 8 token blocks for projections
NDC = D // 128     # 8 contraction chunks

MASK_NEG = -30000.0
SCALE = 0.125
PE_KCS = frozenset((0, 1, 2, 3, 4, 5, 15))   # units bias-injected on PE


def _build_program():
    nc = bacc.Bacc(
        "TRN2", target_bir_lowering=False, debug=False, num_devices=NCORES
    )
    hid3 = nc.dram_tensor("hid3", [128, NSB, NDC, 512], BF16,
                          kind="ExternalInput").ap()
    amask = nc.dram_tensor("attention_mask", [B, S], I32, kind="ExternalInput").ap()
    bias8 = nc.dram_tensor(
        "bias8", [NQB, 128, HPC, NKC, QB], FP8, kind="ExternalInput").ap()
    wqt = nc.dram_tensor("wq_t", [128, NDC, 128], BF16, kind="ExternalInput").ap()
    wkt = nc.dram_tensor("wk_t", [128, NDC, 128], BF16, kind="ExternalInput").ap()
    wvt = nc.dram_tensor("wv_t", [128, NDC, 128], BF16, kind="ExternalInput").ap()
    bq = nc.dram_tensor("bq", [OC], F32, kind="ExternalInput").ap()
    bk = nc.dram_tensor("bk", [OC], F32, kind="ExternalInput").ap()
    bv = nc.dram_tensor("bv", [OC], F32, kind="ExternalInput").ap()
    out = nc.dram_tensor("out", [B, S, OC], F32, kind="ExternalOutput").ap()

    with tile.TileContext(nc) as tc:
        _attention(tc, out, hid3, amask, bias8,
                   [wqt, wkt, wvt], [bq, bk, bv])

    nc.compile()
    return nc


def _attention(tc, out, hid3, amask, bias8, ws, bs):
    nc = tc.nc

    with tc.tile_pool(name="singles", bufs=1) as singles, \
         tc.tile_pool(name="h_t", bufs=3) as htp, \
         tc.tile_pool(name="v_t", bufs=3) as vtp:
        # --- front-load the big phase-1 DMAs (descriptor-cheap layouts) ---
        wt3 = []
        for i, w_ap in enumerate(ws):
            t = singles.tile([128, NDC, 128], BF16, tag=f"wt{i}")
            nc.sync.dma_start(out=t, in_=w_ap)
            wt3.append(t)
        hts_tiles = {}
        for sb in (0, 1):
            hts_tiles[sb] = htp.tile([128, NDC, 512], BF16, name="hts")
            nc.sync.dma_start(out=hts_tiles[sb], in_=hid3[:, sb])
        bvec = []
        for i, b_ap in enumerate(bs):
            t = singles.tile([128, 1], F32, tag=f"bvec{i}")
            nc.gpsimd.dma_start(out=t, in_=b_ap.rearrange("(p o) -> p o", o=1))
            bvec.append(t)
        ones_col = singles.tile([128, 1], BF16)
        nc.vector.memset(ones_col, 1.0)

        # --- persistent activations (bf16) --------------------------------
        qt2 = singles.tile([128, B * S], BF16, tag="qt2")
        kt2 = singles.tile([128, B * S], BF16, tag="kt2")
        va = singles.tile([128, 2 * NKC, 2 * 66], BF16, tag="va")

        pend_vt2 = []

        def emit_hts_dma(sb):
            if sb < NSB and sb not in hts_tiles:
                hts_tiles[sb] = htp.tile([128, NDC, 512], BF16, name="hts")
                nc.sync.dma_start(out=hts_tiles[sb], in_=hid3[:, sb])

        def emit_proj_w(sb, w, pool):
            hts = hts_tiles[sb]
            pp = pool.tile([128, QB], F32, tag="ep", name="pp")
            for dc in range(NDC):
                nc.tensor.matmul(
                    out=pp, lhsT=wt3[w][:, dc, :], rhs=hts[:, dc, :],
                    start=(dc == 0), stop=(dc == NDC - 1))
            if w < 2:
                dst = (qt2 if w == 0 else kt2)[:, sb * 512:(sb + 1) * 512]
                nc.scalar.activation(out=dst, in_=pp, func=Ident, bias=bvec[w])
            else:
                if sb % 2 == 0:
                    vt2 = vtp.tile([128, 2, 512], BF16, name="vt2")
                    pend_vt2.append(vt2)
                else:
                    vt2 = pend_vt2[-1]
                nc.vector.tensor_scalar_add(
                    out=vt2[:, sb % 2, :], in0=pp, scalar1=bvec[2])
                if sb % 2 == 1:
                    vts = vtp.tile([128, 8, 128], BF16, name="vts")
                    nc.sync.dma_start(
                        out=vts, in_=vt2.rearrange("p j q -> p (j q)"),
                        transpose=True)
                    for j in range(8):
                        kb = (sb - 1) * 4 + j
                        for h in range(HPC):
                            nc.gpsimd.tensor_copy(
                                out=va[:, kb, h * 66:h * 66 + 64],
                                in_=vts[:, j, h * 64:(h + 1) * 64])
                            nc.gpsimd.tensor_copy(
                                out=va[:, kb, h * 66 + 64:h * 66 + 65],
                                in_=ones_col)

        # ============ phase 1: b=0 projections ============================
        with tc.tile_pool(name="p_ps", bufs=4, space="PSUM") as pps:
            for sb in range(NSB // 2):
                emit_hts_dma(sb + 2)   # stay 2 ahead
                for w in range(3):
                    emit_proj_w(sb, w, pps)

        # --- phase-2-only setup (emitted late, runs in parallel) ----------
        ident = singles.tile([128, 128], F32)    # for epilogue PE transposes
        make_identity(nc, ident)
        id8 = singles.tile([128, 128], FP8)      # fp8 identity for bias inject
        nc.vector.tensor_copy(out=id8, in_=ident)
        mi = singles.tile([128, B, NKC], I32)
        nc.gpsimd.dma_start(out=mi, in_=amask.rearrange("b (c p) -> p b c", p=128))
        mf = singles.tile([128, B, NKC], F32)
        nc.vector.tensor_copy(out=mf, in_=mi)
        mb = singles.tile([128, B, NKC], F32)
        nc.vector.tensor_scalar(
            out=mb, in0=mf, scalar1=-MASK_NEG, scalar2=MASK_NEG,
            op0=mybir.AluOpType.mult, op1=mybir.AluOpType.add,
        )

        # ============ phase 2: attention (b outer) ========================
        with tc.tile_pool(name="b_t", bufs=2) as btp, \
             tc.tile_pool(name="pt", bufs=3) as ptp, \
             tc.tile_pool(name="se", bufs=3) as sep, \
             tc.tile_pool(name="stage", bufs=3) as stp, \
             tc.tile_pool(name="osb", bufs=3) as osp, \
             tc.tile_pool(name="sc_ps", bufs=2, space="PSUM") as scp, \
             tc.tile_pool(name="ep_ps", bufs=2, space="PSUM") as epp, \
             tc.tile_pool(name="ctx_ps", bufs=2, space="PSUM") as cxp:
            pending = []        # deferred epilogue emitters (1 pop/unit)
            proj_pending = []   # deferred b=1 projections (pop at kc%4==2)

            for sb in range(NSB // 2, NSB):
                def dma_cl(sb=sb):
                    emit_hts_dma(sb + 1)
                proj_pending.append(dma_cl)
                for w in range(3):
                    def proj_cl(sb=sb, w=w):
                        emit_proj_w(sb, w, epp)
                    proj_pending.append(proj_cl)

            def emit_epilogue(ctx, b, qb):
                stage = stp.tile([128, QB], F32, tag="stage", name="stage")
                rst = stp.tile([128, QB], F32, tag="rst", name="rst")
                osb = osp.tile([128, 4, 128], F32, tag="osb", name="osb")

                def head_drain():
                    for h in range(HPC):
                        nc.vector.tensor_copy(
                            out=stage[h * 64:(h + 1) * 64, :],
                            in_=ctx[h][0:64, :])
                        nc.vector.tensor_copy(
                            out=rst[32 * h:32 * h + 1, :],
                            in_=ctx[h][64:65, :])
                pending.append(head_drain)

                def quarter(i):
                    def emit():
                        tp = epp.tile([128, 128], F32, tag="ep", name="ep_t")
                        rp = epp.tile([128, 128], F32, tag="ep", name="ep_r")
                        nc.tensor.transpose(
                            out=tp, in_=stage[:, i * 128:(i + 1) * 128],
                            identity=ident)
                        nc.tensor.transpose(
                            out=rp, in_=rst[:, i * 128:(i + 1) * 128],
                            identity=ident)
                        rcp = stp.tile([128, 2], F32, tag="rcp", name="rcp")
                        for h in range(HPC):
                            nc.vector.reciprocal(
                                out=rcp[:, h:h + 1],
                                in_=rp[:, 32 * h:32 * h + 1])
                            nc.vector.tensor_scalar_mul(
                                out=osb[:, i, h * 64:(h + 1) * 64],
                                in0=tp[:, h * 64:(h + 1) * 64],
                                scalar1=rcp[:, h:h + 1])
                    return emit
                for i in range(4):
                    pending.append(quarter(i))

                def store():
                    nc.gpsimd.dma_start(
                        out=out[b, qb * QB:(qb + 1) * QB, :]
                        .rearrange("(i p) k -> p i k", p=128),
                        in_=osb)
                pending.append(store)

            bt8s = {}

            def load_bt8(qb):
                t = btp.tile([128, HPC, NKC, QB], FP8, tag="bt8",
                             name=f"bt8_{qb}")
                nc.sync.dma_start(out=t, in_=bias8[qb])
                return t

            bt8s[0] = load_bt8(0)
            for b in range(B):
                for qb in range(NQB):
                    bt8 = bt8s.pop(qb)
                    nqb = qb + 1 if qb + 1 < NQB else (0 if b == 0 else None)
                    if nqb is not None:
                        bt8s[nqb] = load_bt8(nqb)
                    ctx = [cxp.tile([65, QB], F32, tag="ctx", name=f"ctx{b}{h}")
                           for h in range(HPC)]
                    prev_pt = None

                    def emit_av(pt, kc, ctx=ctx, b=b):
                        for h in range(HPC):
                            nc.tensor.matmul(
                                out=ctx[h],
                                lhsT=va[:, b * NKC + kc, h * 66:h * 66 + 65],
                                rhs=pt[:, h, :],
                                start=(kc == 0), stop=(kc == NKC - 1))

                    for kc in range(NKC):
                        pe_unit = kc in PE_KCS
                        sc2 = scp.tile([128, HPC, QB], F32, tag="sc", name="sc2")
                        if pe_unit:
                            for h in range(HPC):
                                nc.tensor.matmul(
                                    out=sc2[:, h, :], lhsT=id8,
                                    rhs=bt8[:, h, kc, :],
                                    start=True, stop=False,
                                    skip_group_check=True)
                        for h in range(HPC):
                            nc.tensor.matmul(
                                out=sc2[:, h, :],
                                lhsT=kt2[h * 64:(h + 1) * 64,
                                         b * S + kc * 128:
                                         b * S + (kc + 1) * 128],
                                rhs=qt2[h * 64:(h + 1) * 64,
                                        b * S + qb * QB:
                                        b * S + (qb + 1) * QB],
                                start=not pe_unit, stop=True,
                                tile_position=(h * 64, 0),
                                skip_group_check=True)
                        if pe_unit:
                            esrc = sc2
                        else:
                            esrc = sep.tile([128, HPC, QB], F32,
                                            tag="se", name="se")
                            for h in range(HPC):
                                nc.vector.tensor_tensor(
                                    out=esrc[:, h, :], in0=sc2[:, h, :],
                                    in1=bt8[:, h, kc, :],
                                    op=mybir.AluOpType.add)
                        pt = ptp.tile([128, HPC, QB], BF16, tag="pt", name="pt")
                        nc.scalar.activation(
                            out=pt.rearrange("p h q -> p (h q)"),
                            in_=esrc.rearrange("p h q -> p (h q)"), func=Exp,
                            bias=mb[:, b, kc:kc + 1], scale=SCALE)
                        if prev_pt is not None:
                            emit_av(*prev_pt)
                        if kc % 4 == 2 and proj_pending:
                            proj_pending.pop(0)()
                        elif pending:
                            pending.pop(0)()
                        prev_pt = (pt, kc)
                    emit_av(*prev_pt)
                    emit_epilogue(ctx, b, qb)
            while pending:
                pending.pop(0)()


_CACHE = {}


def _get_program():
    if "nc" not in _CACHE:
        _CACHE["nc"] = _build_program()
    return _CACHE["nc"]


def _shard_inputs(inputs):
    """Host-side layout prep: transposes and dtype casts only (no compute)."""
    bf = ml_dtypes.bfloat16
    fp8 = ml_dtypes.float8_e4m3fn
    hs = np.asarray(inputs["hidden_state"], dtype=np.float32)
    # hid3[p, sb, c, s] = hidden[sb*512+s, c*128+p]  (per-partition 8KB runs)
    hid3 = np.ascontiguousarray(
        hs.reshape(B * S, D).T.reshape(NDC, 128, NSB, 512)
        .transpose(1, 2, 0, 3)).astype(bf)
    am = np.ascontiguousarray(np.asarray(inputs["attention_mask"], dtype=np.int32))
    ab = np.asarray(inputs["attention_bias"], dtype=np.float32)
    wts = {k: np.asarray(inputs[k], dtype=np.float32) for k in ("Wq", "Wk", "Wv")}
    vb = {k: np.ascontiguousarray(np.asarray(inputs[k], dtype=np.float32))
          for k in ("bq", "bk", "bv")}

    def wlay(w):
        # [128, NDC, 128] with partition-contiguous 2KB runs
        return np.ascontiguousarray(
            w.T.reshape(NDC, 128, OC).transpose(1, 0, 2)).astype(bf)

    in_maps = []
    for c in range(NCORES):
        r0, r1 = c * OC, (c + 1) * OC
        # bias8[qb, p, h, kc, q] = ab[0, 2c+h, qb*QB+q, kc*128 + p]
        x = ab[0, HPC * c:HPC * (c + 1)]           # [h, q(S), k(S)]
        x = x.transpose(2, 0, 1)                   # [k, h, q]
        x = x.reshape(NKC, 128, HPC, NQB, QB)      # [kc, p, h, qb, q]
        x = x.transpose(3, 1, 2, 0, 4)             # [qb, p, h, kc, q]
        b8 = np.ascontiguousarray(x).astype(fp8)
        in_maps.append({
            "hid3": hid3,
            "attention_mask": am,
            "bias8": b8,
            "wq_t": wlay(wts["Wq"][r0:r1]),
            "wk_t": wlay(wts["Wk"][r0:r1]),
            "wv_t": wlay(wts["Wv"][r0:r1]),
            "bq": vb["bq"][r0:r1],
            "bk": vb["bk"][r0:r1],
            "bv": vb["bv"][r0:r1],
        })
    return in_maps


def kernel(**inputs):
    nc = _get_program()
    in_maps = _shard_inputs(inputs)
    res = bass_utils.run_bass_kernel_spmd(
        nc, in_maps, core_ids=list(range(NCORES)))
    parts = [np.asarray(res.results[c]["out"]) for c in range(NCORES)]
    return np.concatenate(parts, axis=-1)


def run_profiled(inputs, trace=True):
    """test.py helper: returns (output, BassKernelResults)."""
    nc = _get_program()
    in_maps = _shard_inputs(inputs)
    res = bass_utils.run_bass_kernel_spmd(
        nc, in_maps, core_ids=list(range(NCORES)), trace=trace)
    parts = [np.asarray(res.results[c]["out"]) for c in range(NCORES)]
    return np.concatenate(parts, axis=-1), res
